# revision 1
# baseline (speedup 1.0000x reference)
"""ConditionalConv Trainium2 kernel (8 NeuronCores, SPMD).

Reference computation (per sample b):
    w_b = tanh(conditioning @ W_cond.T + b_cond) * 5        [B, 36928]
    bias = w_b[:, -64:]; w = w_b[:, :-64].reshape(B, 64, 64, 3, 3)
    y[b] = conv2d(x[b], w[b], pad=1) + bias[b]

Strategy:
  - Data-parallel conv: 2 samples per core (batch shard).
  - Hypernetwork sharded over the 36864 weight-params (4608/core, exactly
    9 N=512 matmul tiles); the 64 conv-bias params are computed by every
    core (replicated). Host pre-permutes W_cond rows to tap-major order
    (tap, ic, oc) and pre-transposes, so each core's slice streams as
    contiguous rhs tiles and the AllToAll output yields DMA-contiguous
    [ic, oc] conv-weight tiles.
  - The Linear bias b_cond is folded in as an extra contraction row
    against a constant ones row appended to conditioning^T (zero-padded
    to a full 128-row K-chunk to keep one PE tiling mode).
  - tanh on ACT during PSUM evacuation; the final x5 of the reference is
    folded into the conv output evacuation: y = 5*(conv(x,tanh_w)+tanh_b).
  - AllToAll redistributes the per-param-slice hypernet output to the
    per-sample owner cores with static addressing.
  - Conv: per-sample host-zero-padded fp16 input staged as [128, 130x130]
    where partitions 64-127 hold the image shifted down one row, so taps
    (kh=0,kw) and (kh=1,kw) pack into one K=128 contraction: 3 K=128 +
    3 K=64 accumulating matmuls per N=512 pixel tile (vs 9 K=64) --
    back-to-back PE matmul streams do not overlap across array tiles on
    HW, so fewer/fuller streams is what cuts PE time.
"""

import numpy as np
from contextlib import ExitStack

import concourse.bacc as bacc
import concourse.tile as tile
import concourse.mybir as mybir
from concourse.bass_utils import run_bass_kernel_spmd

dt = mybir.dt
AF = mybir.ActivationFunctionType
ALU = mybir.AluOpType

N_CORES = 8
B, COND_C = 16, 256
IN_C, OUT_C, KS = 64, 64, 3
H = W = 128
NW = KS * KS * IN_C * OUT_C          # 36864 weight params
N_PARAM = NW + OUT_C                 # 36928
SLICE = NW // N_CORES                # 4608 params per core
HSN = SLICE + OUT_C                  # 4672 hypernet outputs per core
S = B // N_CORES                     # 2 samples per core
HP = H + 2                           # 130 padded
PT = 32                              # pixel tiles (4 output rows each)
KCH = 3                              # hypernet contraction chunks of 128

_cache = {}


def _build(repeat_conv=1, loop=0):
    """Build + compile the 8-core SPMD bass program."""
    nc = bacc.Bacc("TRN2", target_bir_lowering=False, debug=False,
                   num_devices=N_CORES)

    r = dt.float32r
    xs = nc.dram_tensor("xs", [S, IN_C, HP, HP], dt.float16, kind="ExternalInput").ap()
    hs = nc.dram_tensor("hs", [COND_C + 1, HSN], dt.float16, kind="ExternalInput").ap()
    ct = nc.dram_tensor("ct", [COND_C + 1, B], dt.float16, kind="ExternalInput").ap()
    ys = nc.dram_tensor("ys", [S, OUT_C, H, W], dt.float32, kind="ExternalOutput").ap()

    hyp_out = nc.dram_tensor("hyp_out", [B, HSN], dt.float32, kind="Internal")
    hyp_rcv = nc.dram_tensor("hyp_rcv", [B, HSN], dt.float32, kind="Internal")

    with tile.TileContext(nc) as tc:
        with ExitStack() as ctx:
            cpool = ctx.enter_context(tc.tile_pool(name="consts", bufs=1))
            hpool = ctx.enter_context(tc.tile_pool(name="hyp", bufs=3))
            epool = ctx.enter_context(tc.tile_pool(name="evac", bufs=3))
            ppool = ctx.enter_context(tc.tile_pool(name="psum", bufs=2, space="PSUM"))

            # ---------------- tile allocs ----------------
            # K chunks of 86/86/85 (all round up to the 128-row PE tiling
            # mode, so no zero padding is needed for the 257th ones-row).
            CH = [(0, 86), (86, 172), (172, COND_C + 1)]
            cts = [cpool.tile([128, B], dt.float16, name=f"ct{k}")
                   for k in range(KCH)]
            hss = [cpool.tile([128, HSN], dt.float16, name=f"hs{k}")
                   for k in range(KCH)]
            # Per-sample input: partitions (j, ic) with j in {0,1}; the upper
            # half holds the image shifted down one row, so taps kh=0 and
            # kh=1 pack into one K=128 contraction.
            xdup = [cpool.tile([128, HP * HP], dt.float16, name=f"xdup{s}")
                    for s in range(S)]
            xdv = [xdup[s][:].rearrange("p (h w) -> p h w", w=HP)
                   for s in range(S)]

            loop_cm = (tc.For_i(0, loop, 1,
                                hint_engines=(mybir.EngineType.PE,))
                       if loop else None)
            if loop_cm is not None:
                loop_cm.__enter__()

            # ---------------- input loads (phase 1) ----------------
            for k, (klo, khi) in enumerate(CH):
                nc.sync.dma_start(cts[k][0:khi - klo, :], ct[klo:khi, :])
                nc.sync.dma_start(hss[k][0:khi - klo, :], hs[klo:khi, :])

            # ---------------- hypernetwork ----------------
            # out[b, p] = sum_c cond[b, c] * Wp[p, c] (+ b_cond via ones row),
            # tanh on evacuation.
            with nc.named_scope("hyper"):
                for j in range(10):
                    n0 = j * 512
                    nn = 512 if j < 9 else OUT_C
                    hp = ppool.tile([B, nn], dt.float32, name=f"hp{j}",
                                    tag=f"acc{j % 4}")
                    for k, (klo, khi) in enumerate(CH):
                        nc.tensor.matmul(hp[:], cts[k][0:khi - klo, :],
                                         hss[k][0:khi - klo, n0:n0 + nn],
                                         start=(k == 0), stop=(k == KCH - 1))
                    tht = hpool.tile([B, nn], dt.float32, name=f"th{j}", tag="th")
                    nc.scalar.activation(tht[:], hp[:], AF.Tanh)
                    nc.sync.dma_start(hyp_out.ap()[:, n0:n0 + nn], tht[:])

            if loop_cm is not None:
                loop_cm.__exit__(None, None, None)

            # ---------------- redistribute ----------------
            with nc.named_scope("cc"):
                nc.gpsimd.collective_compute(
                    "AllToAll", ALU.bypass,
                    replica_groups=[list(range(N_CORES))],
                    ins=[hyp_out.ap()], outs=[hyp_rcv.ap()],
                )

            loop_cm2 = (tc.For_i(0, loop, 1,
                                 hint_engines=(mybir.EngineType.PE,))
                        if loop else None)
            if loop_cm2 is not None:
                loop_cm2.__enter__()

            # ---------------- input loads (phase 2) ----------------
            for s in range(S):
                xsf = xs[s].rearrange("c h w -> c (h w)")
                nc.sync.dma_start(xdup[s][0:64, :], xsf)
                nc.sync.dma_start(xdup[s][64:128, 0:(HP - 1) * HP],
                                  xsf[:, HP:HP * HP])

            # ---------------- conv weight tiles ----------------
            # hyp_rcv row k*S+s = (my sample s)'s params [k*4608, (k+1)*4608).
            # Permuted param index n = tap*4096 + ic*64 + oc.
            hv = hyp_rcv.ap().rearrange("b (p q) -> b p q", q=64)  # [16, 73, 64]
            # wpair[s][kw]: partitions 0-63 = tap (0,kw), 64-127 = tap (1,kw)
            # wrem[s][kw]:  partitions 0-63 = tap (2,kw)
            wpair = [[cpool.tile([128, 64], dt.float16, name=f"wp{s}_{kw}")
                      for kw in range(KS)] for s in range(S)]
            wrem = [[cpool.tile([128, 64], dt.float16, name=f"wr{s}_{kw}")
                     for kw in range(KS)] for s in range(S)]
            with nc.named_scope("wload"):
                for t in range(KS * KS):
                    kh, kw = divmod(t, KS)
                    lo, hi = t * 4096, (t + 1) * 4096
                    cuts = [lo] + [m for m in range(SLICE, NW, SLICE)
                                   if lo < m < hi] + [hi]
                    for s in range(S):
                        tile_t = wpair[s][kw] if kh < 2 else wrem[s][kw]
                        base = 64 * kh if kh < 2 else 0
                        for a, b_ in zip(cuts[:-1], cuts[1:]):
                            k = a // SLICE
                            src = hv[k * S + s,
                                     (a - k * SLICE) // 64:(b_ - k * SLICE) // 64, :]
                            dst = tile_t[base + (a - lo) // 64:
                                         base + (b_ - lo) // 64, :]
                            nc.gpsimd.dma_start(dst, src)  # casts f32 -> f16
                # conv bias columns: per-partition layout of tanh_bias.
                # tbA: parts 0-63 = sample0, 64-127 = sample1 (for q0/q3)
                # tbB: parts 0-63 = sample1, 64-127 = sample0 (for q2/q1)
                tbA = cpool.tile([128, 1], dt.float32)
                tbB = cpool.tile([128, 1], dt.float32)
                for s in range(S):
                    src = hyp_rcv.ap()[s, SLICE:SLICE + OUT_C]
                    nc.sync.dma_start(tbA[s * 64:(s + 1) * 64, :], src)
                    nc.sync.dma_start(tbB[(1 - s) * 64:(2 - s) * 64, :], src)
                tbA5 = cpool.tile([128, 1], dt.float32)
                tbB5 = cpool.tile([128, 1], dt.float32)
                nc.scalar.activation(tbA5[:], tbA[:], AF.Copy, scale=5.0)
                nc.scalar.activation(tbB5[:], tbB[:], AF.Copy, scale=5.0)

            # ---------------- conv ----------------
            ysv = ys.rearrange("s c (j v) w -> s c j (v w)", v=4)  # [S,64,32,512]
            with nc.named_scope("conv"):
                for rep in range(repeat_conv):
                    for jj in range(PT // 2):
                        j0, j1 = 2 * jj, 2 * jj + 1
                        # quadrant q -> (sample, ptile, psum col base)
                        quads = [(0, j0, 0), (0, j1, 64), (1, j0, 0), (1, j1, 64)]
                        accs = [ppool.tile([128, 512], dt.float32,
                                           name=f"cp{rep}_{jj}_{q}", tag=f"acc{q}")
                                for q in range(4)]
                        for m in range(2 * KS):
                            kw, is_rem = m % KS, m >= KS
                            for q, (s, j, cb) in enumerate(quads):
                                h0 = 4 * j
                                if not is_rem:
                                    # taps (0,kw)+(1,kw), K=128
                                    nc.tensor.matmul(
                                        accs[q][cb:cb + 64, :],
                                        wpair[s][kw][:, :],
                                        xdv[s][:, h0:h0 + 4, kw:kw + 128],
                                        start=(m == 0), stop=False)
                                else:
                                    # tap (2,kw), K=64
                                    nc.tensor.matmul(
                                        accs[q][cb:cb + 64, :],
                                        wrem[s][kw][0:64, :],
                                        xdv[s][0:64, h0 + 2:h0 + 6, kw:kw + 128],
                                        start=False, stop=(m == 2 * KS - 1))
                        # evacuation: y = 5*psum + 5*tanh_bias
                        yoE = epool.tile([128, 512], dt.float32,
                                         name=f"yoE{rep}_{jj}", tag="yoE")
                        yoO = epool.tile([128, 512], dt.float32,
                                         name=f"yoO{rep}_{jj}", tag="yoO")
                        # q0 (s0,j0) psum[0:64]  -> yoE[0:64]   (ACT)
                        nc.scalar.activation(yoE[0:64, :], accs[0][0:64, :],
                                             AF.Identity, bias=tbA5[0:64, :],
                                             scale=5.0)
                        # q3 (s1,j1) psum[64:128] -> yoE[64:128] (DVE)
                        nc.vector.tensor_scalar(yoE[64:128, :], accs[3][64:128, :],
                                                5.0, tbA5[64:128, :],
                                                ALU.mult, ALU.add)
                        # q2 (s1,j0) psum[0:64]  -> yoO[0:64]   (DVE)
                        nc.vector.tensor_scalar(yoO[0:64, :], accs[2][0:64, :],
                                                5.0, tbB5[0:64, :],
                                                ALU.mult, ALU.add)
                        # q1 (s0,j1) psum[64:128] -> yoO[64:128] (ACT)
                        nc.scalar.activation(yoO[64:128, :], accs[1][64:128, :],
                                             AF.Identity, bias=tbB5[64:128, :],
                                             scale=5.0)
                        nc.sync.dma_start(ysv[0, :, j0, :], yoE[0:64, :])
                        nc.sync.dma_start(ysv[1, :, j1, :], yoE[64:128, :])
                        nc.sync.dma_start(ysv[1, :, j0, :], yoO[0:64, :])
                        nc.sync.dma_start(ysv[0, :, j1, :], yoO[64:128, :])

            if loop_cm2 is not None:
                loop_cm2.__exit__(None, None, None)

    nc.compile()
    return nc


def _prep_inputs(x, conditioning, W_cond, b_cond):
    """Host-side shard + permute. Returns per-core input maps."""
    x = np.asarray(x, dtype=np.float32)
    conditioning = np.asarray(conditioning, dtype=np.float32)
    W_cond = np.asarray(W_cond, dtype=np.float32)
    b_cond = np.asarray(b_cond, dtype=np.float32)

    t = np.arange(KS * KS)
    i = np.arange(IN_C)
    o = np.arange(OUT_C)
    # permuted n = (tap, ic, oc) -> original p = oc*576 + ic*9 + tap
    perm = (o[None, None, :] * (IN_C * KS * KS) + i[None, :, None] * (KS * KS)
            + t[:, None, None]).reshape(-1)
    Wp = W_cond[perm]                      # [36864, 256]
    bp = b_cond[perm]

    # [257, 36864]: rows 0-255 = Wp^T, row 256 = bp (ones-row bias fold)
    AaugW = np.zeros((COND_C + 1, NW), np.float16)
    AaugW[0:COND_C] = Wp.T.astype(np.float16)
    AaugW[COND_C] = bp.astype(np.float16)
    AaugB = np.zeros((COND_C + 1, OUT_C), np.float16)
    AaugB[0:COND_C] = W_cond[NW:].T.astype(np.float16)
    AaugB[COND_C] = b_cond[NW:].astype(np.float16)

    ctaug = np.zeros((COND_C + 1, B), np.float16)
    ctaug[0:COND_C] = conditioning.T.astype(np.float16)
    ctaug[COND_C] = 1.0

    xpadded = np.zeros((B, IN_C, HP, HP), np.float16)
    xpadded[:, :, 1:HP - 1, 1:HP - 1] = x.astype(np.float16)

    in_maps = []
    for c in range(N_CORES):
        hs_c = np.ascontiguousarray(
            np.concatenate([AaugW[:, c * SLICE:(c + 1) * SLICE], AaugB], axis=1),
            dtype=np.float16)
        xs_c = np.ascontiguousarray(xpadded[c * S:(c + 1) * S])  # float16
        in_maps.append({"xs": xs_c, "hs": hs_c, "ct": ctaug})
    return in_maps


def _get_nc(repeat_conv=1, loop=0):
    key = (repeat_conv, loop)
    if key not in _cache:
        _cache[key] = _build(repeat_conv, loop)
    return _cache[key]


def _assemble(results):
    return np.concatenate([results[c]["ys"] for c in range(N_CORES)], axis=0)


def kernel(x, conditioning, W_cond, b_cond):
    nc = _get_nc()
    in_maps = _prep_inputs(x, conditioning, W_cond, b_cond)
    res = run_bass_kernel_spmd(nc, in_maps, list(range(N_CORES)))
    return _assemble(res.results)


# ---- helpers for the local test harness (not used by the grader) ----

def run_sim(x, conditioning, W_cond, b_cond):
    import concourse.bass_interp as bass_interp

    nc = _get_nc()
    in_maps = _prep_inputs(x, conditioning, W_cond, b_cond)
    sim = bass_interp.MultiCoreSim(nc, N_CORES)
    for c in range(N_CORES):
        for k, v in in_maps[c].items():
            sim.cores[c].tensor(k)[:] = v
    sim.simulate()
    results = [{"ys": np.array(sim.cores[c].tensor("ys"))} for c in range(N_CORES)]
    return _assemble(results)

